# revision 17
# baseline (speedup 1.0000x reference)
"""Trainium2 Bass kernel for nn_EnvironmentSpecificDecoder.

Data-parallel over batch B=32 across 8 NeuronCores (4 batches/core).
Per (b,t) slice (z [d=128, L=64]):
  stage1 : pair-packed fp32r matmuls: lhsT=[Zs_t0|Zs_t1] vs rhs=A gives
           z_aggT for two t's at once ([0:64]=t0, [64:128]=t1); the same
           with lhsT=[Zc_t0|Zc_t1] vs rhs=I transposes z_corrupt.
  S23    : fused signal projection + env MLP layer1 (W1s = W_sig @ W1[e],
           host-precomputed per env, dispatched on-device by regime via
           dynamic-offset DMA): h1T[h,(t,i)] = relu(W1s^T z_aggT + b1s).
  C1     : corrupt path h_cT = relu(Wc^T Zc^T + bc).
  S4+C2  : out2T[k,(t,i)] = W2[e]^T h1T  (+ Wo^T h_cT accumulated into the
           mu row of the same PSUM bank; bo+b2 folded into biases).
  post   : thin [2,512] evacuation per quad, SBUF->SBUF DMA compaction to
           dense [64, 512] tiles, softplus = ln(exp(x)+1), +0.01, then two
           strided output DMAs.

All matmuls run in float32r (E8M11): full fp32 range, 12-bit significand,
exact fp32 PSUM accumulation. Inputs are pre-rounded host-side.
"""
import os
import numpy as np

N_CORES = 8
NB = 4          # batches per core
T = 64
D = 128
L = 64
H = 256
H2 = 128
NE = 8

_CACHE = {}


def _round_fp32r(x: np.ndarray) -> np.ndarray:
    """Round fp32 array to E8M11 (float32r) with round-to-nearest-even."""
    u = np.ascontiguousarray(x, dtype=np.float32).view(np.uint32)
    keep = np.uint32(12)
    half = np.uint32(1 << 11)
    lsb = (u >> keep) & np.uint32(1)
    return ((u + (half - np.uint32(1) + lsb)) >> keep << keep).view(np.float32)


def _build():
    import concourse.bacc as bacc
    import concourse.bass as bass
    import concourse.mybir as mybir
    from concourse.tile import TileContext

    F32 = mybir.dt.float32
    F32R = mybir.dt.float32r
    AF = mybir.ActivationFunctionType
    ADD = mybir.AluOpType.add
    MAX = mybir.AluOpType.max

    nc = bacc.Bacc("TRN2", target_bir_lowering=False, debug=False)

    # zzi: [b, j, path(s/c), pair, t01*L]  (pair-packed lhsT slices)
    zzi_d = nc.dram_tensor("zzi", [NB, D, T * 2 * L], F32R, kind="ExternalInput")
    ai_d = nc.dram_tensor("ai", [D, 2 * D], F32R, kind="ExternalInput")
    reg_d = nc.dram_tensor("reg", [1, NB], mybir.dt.int32, kind="ExternalInput")
    w1s_d = nc.dram_tensor("w1s", [NE, D, H], F32R, kind="ExternalInput")
    b1s_d = nc.dram_tensor("b1s", [NE, D, 2], F32, kind="ExternalInput")
    w2p_d = nc.dram_tensor("w2p", [NE, D, 2, 2], F32R, kind="ExternalInput")
    b2a_d = nc.dram_tensor("b2a", [NE, 2, 1], F32, kind="ExternalInput")
    wc_d = nc.dram_tensor("wc", [D, H2], F32R, kind="ExternalInput")
    bc_d = nc.dram_tensor("bc", [H2, 1], F32, kind="ExternalInput")
    wo_d = nc.dram_tensor("wo", [H2, 1], F32R, kind="ExternalInput")

    mu_d = nc.dram_tensor("mu", [NB, T, D], F32, kind="ExternalOutput")
    sg_d = nc.dram_tensor("sg", [NB, T, D], F32, kind="ExternalOutput")

    with TileContext(nc) as tc:
        with (
            tc.tile_pool(name="const", bufs=1) as constp,
            tc.tile_pool(name="zz", bufs=2) as zzp,
            tc.tile_pool(name="ev", bufs=3) as evp,
            tc.tile_pool(name="stg", bufs=2) as stgp,
            tc.tile_pool(name="fin", bufs=1) as finp,
            tc.tile_pool(name="ps1", bufs=2, space="PSUM") as ps1,
            tc.tile_pool(name="ps23", bufs=1, space="PSUM") as ps23,
            tc.tile_pool(name="psc", bufs=1, space="PSUM") as psc,
            tc.tile_pool(name="ps4", bufs=1, space="PSUM") as ps4,
        ):
            # ---- static weights ----
            ai_sb = constp.tile([D, 2 * D], F32R)
            nc.sync.dma_start(ai_sb[:], ai_d[:])
            wc_sb = constp.tile([D, H2], F32R)       # Wc stacked twice
            nc.sync.dma_start(wc_sb[:], wc_d[:])
            wo_sb = constp.tile([H2, 1], F32R)
            nc.sync.dma_start(wo_sb[:], wo_d[:])
            bc_sb = constp.tile([H2, 1], F32)
            nc.sync.dma_start(bc_sb[:], bc_d[:])
            reg_sb = constp.tile([1, NB], mybir.dt.int32)
            nc.sync.dma_start(reg_sb[:], reg_d[:])

            # ---- per-batch dispatched weights (regime -> env) ----
            w1s_sb, b1s_sb, w2_sb, b2_sb = [], [], [], []
            for b in range(NB):
                e = nc.values_load(
                    reg_sb[0:1, b : b + 1],
                    engines=[mybir.EngineType.SP],
                    min_val=0, max_val=NE - 1,
                    skip_runtime_bounds_check=True,
                )
                w1 = constp.tile([D, H], F32R, name=f"w1s{b}", tag=f"w1s{b}")
                nc.sync.dma_start(
                    w1[:], w1s_d[bass.ds(e, 1)].rearrange("o p h -> (o p) h")
                )
                b1 = constp.tile([D, 2], F32, name=f"b1s{b}", tag=f"b1s{b}")
                nc.sync.dma_start(
                    b1[:], b1s_d[bass.ds(e, 1)].rearrange("o p h -> (o p) h")
                )
                w2 = constp.tile([D, 2, 2], F32R, name=f"w2{b}", tag=f"w2{b}")
                nc.sync.dma_start(
                    w2[:], w2p_d[bass.ds(e, 1)].rearrange("o p a k -> (o p) a k")
                )
                b2 = constp.tile([2, 1], F32, name=f"b2{b}", tag=f"b2{b}")
                nc.sync.dma_start(
                    b2[:], b2a_d[bass.ds(e, 1)].rearrange("o p k -> (o p) k")
                )
                w1s_sb.append(w1)
                b1s_sb.append(b1)
                w2_sb.append(w2)
                b2_sb.append(b2)

            st_mu = finp.tile([NB * 16, 512], F32)
            st_sig = finp.tile([NB * 16, 512], F32)

            dbg_stages = int(os.environ.get('DBG_STAGES', 4))
            dbg_nb = int(os.environ.get('DBG_NB', NB))
            dbg_ng = int(os.environ.get('DBG_NG', 4))
            for b in range(dbg_nb):
                zz = zzp.tile([D, T * 2 * L], F32R, tag="zz")
                nc.sync.dma_start(zz[:], zzi_d[b])

                for g in range(dbg_ng):
                    st_raw = stgp.tile([2, 2048], F32, tag="st_raw")
                    for q in range(4):
                        qb = g * 4 + q
                        # ---- stage 1: 2 signal pairs + 2 corrupt pairs ----
                        p1 = ps1.tile([D, 512], F32, tag="p1")
                        for tp in range(2):
                            pr = qb * 2 + tp
                            nc.tensor.matmul(
                                p1[:, 128 * tp : 128 * (tp + 1)],
                                zz[:, 128 * pr : 128 * (pr + 1)],
                                ai_sb[:, 0:128],
                                start=True, stop=True,
                            )
                            nc.tensor.matmul(
                                p1[:, 256 + 128 * tp : 256 + 128 * (tp + 1)],
                                zz[:, 4096 + 128 * pr : 4096 + 128 * (pr + 1)],
                                ai_sb[:, 128:256],
                                start=True, stop=True,
                            )
                        # ---- stage-1 evacuation ----
                        zzt = evp.tile([D, 512], F32R, tag="zzt")
                        nc.vector.tensor_copy(zzt[:], p1[:])
                        # layout: cols 0:256 z_aggT (tp, i), 256:512 ZcT;
                        # rows 0:64 even-t of pair, 64:128 odd-t.

                        if dbg_stages < 2:
                            nc.scalar.activation(
                                st_raw[:, 512 * q : 512 * (q + 1)],
                                p1[0:2, :], AF.Identity,
                                bias=b2_sb[b][:, 0:1],
                            )
                            continue
                        # ---- S23: fused L->H layer ----
                        # per-parity banks (mixed row-group matmuls in one
                        # PSUM bank crash the exec unit): col = par*512+hh*256
                        p23 = ps23.tile([D, 1024], F32, tag="p23")
                        for par in range(2):
                            for hh in range(2):
                                nc.tensor.matmul(
                                    p23[:, 512 * par + 256 * hh :
                                        512 * par + 256 * hh + 256],
                                    w1s_sb[b][64 * par : 64 * par + 64,
                                              128 * hh : 128 * (hh + 1)],
                                    zzt[64 * par : 64 * par + 64, 0:256],
                                    start=True, stop=True,
                                )
                        # h1 layout: col = hh*512 + par*256 + tp*128 + i
                        h1 = evp.tile([D, 1024], F32R, tag="h1")
                        p23v = p23[:].rearrange("p (par c) -> p par c", par=2)
                        h1v = h1[:].rearrange("p (hh c) -> p hh c", hh=2)
                        nc.scalar.activation(
                            h1v[:, 0, :].rearrange("p (par c) -> p par c", par=2),
                            p23v[:, :, 0:256],
                            AF.Relu, bias=b1s_sb[b][:, 0:1],
                        )
                        nc.vector.tensor_scalar(
                            h1v[:, 1, :].rearrange("p (par c) -> p par c", par=2),
                            p23v[:, :, 256:512],
                            b1s_sb[b][:, 1:2], 0.0, ADD, MAX,
                        )

                        if dbg_stages < 3:
                            nc.scalar.activation(
                                st_raw[:, 512 * q : 512 * (q + 1)],
                                h1[0:2, 0:512], AF.Identity,
                                bias=b2_sb[b][:, 0:1],
                            )
                            continue
                        # ---- C1: corrupt path ----
                        pc = psc.tile([D, 1024], F32, tag="pc")
                        for par in range(2):
                            nc.tensor.matmul(
                                pc[:, 512 * par : 512 * par + 256],
                                wc_sb[64 * par : 64 * par + 64, :],
                                zzt[64 * par : 64 * par + 64, 256:512],
                                start=True, stop=True,
                            )
                        hc = evp.tile([D, 512], F32R, tag="hc")
                        pcv = pc[:].rearrange("p (par c) -> p par c", par=2)
                        if q % 2 == 0:
                            nc.scalar.activation(
                                hc[:].rearrange("p (par c) -> p par c", par=2),
                                pcv[:, :, 0:256],
                                AF.Relu, bias=bc_sb[:, 0:1],
                            )
                        else:
                            nc.vector.tensor_scalar(
                                hc[:].rearrange("p (par c) -> p par c", par=2),
                                pcv[:, :, 0:256],
                                bc_sb[:, 0:1], 0.0, ADD, MAX,
                            )

                        if dbg_stages < 4:
                            nc.scalar.activation(
                                st_raw[:, 512 * q : 512 * (q + 1)],
                                hc[0:2, :], AF.Identity,
                                bias=b2_sb[b][:, 0:1],
                            )
                            continue
                        # ---- S4 + C2 (contrib accumulated into mu row) ----
                        # rhs cols reordered (par,tp,i) -> (tp,par,i) so p4
                        # columns are t-ordered within the quad.
                        p4 = ps4.tile([2, 512], F32, tag="p4")
                        nc.tensor.matmul(
                            p4[0:2, :],
                            w2_sb[b][:, 0, :],
                            h1[:, 0:512].rearrange(
                                "p (par tp i) -> p par tp i",
                                par=2, tp=2).transpose([0, 2, 1, 3]),
                            start=True, stop=False,
                        )
                        nc.tensor.matmul(
                            p4[0:1, :], wo_sb[:],
                            hc[:].rearrange(
                                "p (par tp i) -> p par tp i",
                                par=2, tp=2).transpose([0, 2, 1, 3]),
                            start=False, stop=False,
                        )
                        nc.tensor.matmul(
                            p4[0:2, :],
                            w2_sb[b][:, 1, :],
                            h1[:, 512:1024].rearrange(
                                "p (par tp i) -> p par tp i",
                                par=2, tp=2).transpose([0, 2, 1, 3]),
                            start=False, stop=True,
                        )
                        nc.scalar.activation(
                            st_raw[:, 512 * q : 512 * (q + 1)], p4[:],
                            AF.Identity, bias=b2_sb[b][:, 0:1],
                        )

                    # ---- compaction: 4 quads -> dense rows ----
                    r0 = b * 16 + g * 4
                    for q in range(4):
                        nc.sync.dma_start(
                            st_mu[r0 + q : r0 + q + 1, :],
                            st_raw[0:1, 512 * q : 512 * (q + 1)],
                        )
                        nc.sync.dma_start(
                            st_sig[r0 + q : r0 + q + 1, :],
                            st_raw[1:2, 512 * q : 512 * (q + 1)],
                        )

            # ---- sigma: softplus + 0.01 (dense) ----
            ex = finp.tile([NB * 16, 512], F32)
            nc.scalar.activation(ex[:], st_sig[:], AF.Exp)
            nc.scalar.activation(st_sig[:], ex[:], AF.Ln, bias=1.0)
            nc.vector.tensor_scalar_add(st_sig[:], st_sig[:], 0.01)

            # ---- outputs ----
            nc.sync.dma_start(
                mu_d[:].rearrange("b (qb tq) i -> (b qb) tq i", tq=4),
                st_mu[:].rearrange("p (tq i) -> p tq i", i=D),
            )
            nc.sync.dma_start(
                sg_d[:].rearrange("b (qb tq) i -> (b qb) tq i", tq=4),
                st_sig[:].rearrange("p (tq i) -> p tq i", i=D),
            )

    nc.compile()
    return nc


def _get_nc():
    if "nc" not in _CACHE:
        _CACHE["nc"] = _build()
    return _CACHE["nc"]


def _prepare_in_maps(z_signal, z_corrupt, A, regime, W_sig, b_sig, W1e, b1e,
                     W2e, b2e, Wc, bc, Wo, bo):
    z_signal = np.asarray(z_signal, dtype=np.float32)
    z_corrupt = np.asarray(z_corrupt, dtype=np.float32)
    A = np.asarray(A, dtype=np.float32)
    regime = np.asarray(regime)
    W_sig = np.asarray(W_sig, dtype=np.float32)
    b_sig = np.asarray(b_sig, dtype=np.float32)
    W1e = np.asarray(W1e, dtype=np.float32)
    b1e = np.asarray(b1e, dtype=np.float32)
    W2e = np.asarray(W2e, dtype=np.float32)
    b2e = np.asarray(b2e, dtype=np.float32)
    Wc = np.asarray(Wc, dtype=np.float32)
    bc = np.asarray(bc, dtype=np.float32)
    Wo = np.asarray(Wo, dtype=np.float32)
    bo = np.asarray(bo, dtype=np.float32)

    eidx = np.where(regime >= NE, 0, regime).astype(np.int32)

    # ---- host weight transforms (env tables, replicated to all cores) ----
    ai = _round_fp32r(np.concatenate([A, np.eye(D, dtype=np.float32)], axis=1))
    w1s_half = _round_fp32r(np.einsum("lh,ehk->elk", W_sig, W1e))  # [E, L, H]
    w1s = np.concatenate([w1s_half, w1s_half], axis=1)             # [E, D, H]
    b1s_full = np.einsum("h,ehk->ek", b_sig, W1e) + b1e            # [E, H]
    b1s = np.ascontiguousarray(
        b1s_full.reshape(NE, 2, D).transpose(0, 2, 1))             # [E, D, 2]
    w2p = _round_fp32r(
        np.ascontiguousarray(
            W2e.reshape(NE, 2, D, 2).transpose(0, 2, 1, 3)))       # [E, D, 2, 2]
    b2a = np.stack([b2e[:, 0] + bo[0], b2e[:, 1]], axis=1)[..., None]  # [E,2,1]
    wc_r = np.concatenate([_round_fp32r(Wc)] * 2, axis=0)          # [D, H2]
    wo_r = _round_fp32r(Wo)                                        # [H2, 1]
    bc_r = np.ascontiguousarray(bc[:, None])                       # [H2, 1]

    in_maps = []
    for c in range(N_CORES):
        b0 = c * NB
        zs = z_signal[b0 : b0 + NB]
        zc = z_corrupt[b0 : b0 + NB]
        # [nb, T, D, L] -> [nb, D, T/2, 2*L] pair-packed, stacked (sig, cor)
        def pack(z):
            zt = z.transpose(0, 2, 1, 3)                 # [nb, D, T, L]
            return zt.reshape(NB, D, T // 2, 2 * L)      # pairs
        zzi = np.concatenate([pack(zs), pack(zc)], axis=2)  # [nb, D, T, 2L]
        zzi = _round_fp32r(np.ascontiguousarray(
            zzi.reshape(NB, D, T * 2 * L)))
        in_maps.append({
            "zzi": zzi,
            "ai": ai,
            "reg": eidx[None, b0 : b0 + NB],
            "w1s": w1s,
            "b1s": b1s,
            "w2p": w2p,
            "b2a": b2a,
            "wc": wc_r,
            "bc": bc_r,
            "wo": wo_r,
        })
    return in_maps


def kernel(z_signal, z_corrupt, A, regime, W_sig, b_sig, W1e, b1e, W2e, b2e,
           Wc, bc, Wo, bo):
    from concourse.bass_utils import run_bass_kernel_spmd

    in_maps = _prepare_in_maps(z_signal, z_corrupt, A, regime, W_sig, b_sig,
                               W1e, b1e, W2e, b2e, Wc, bc, Wo, bo)
    nc = _get_nc()
    res = run_bass_kernel_spmd(nc, in_maps, core_ids=list(range(N_CORES)))

    mu = np.concatenate([r["mu"] for r in res.results], axis=0)
    sigma = np.concatenate([r["sg"] for r in res.results], axis=0)
    return mu, sigma


def run_traced(inputs_np):
    from concourse.bass_utils import run_bass_kernel_spmd

    in_maps = _prepare_in_maps(**inputs_np)
    nc = _get_nc()
    return run_bass_kernel_spmd(
        nc, in_maps, core_ids=list(range(N_CORES)), trace=True
    )
